# revision 1
# baseline (speedup 1.0000x reference)
"""Trainium2 Bass kernel for nn_DifferentiableTMO (histogram_binning).

Strategy: data-parallel over the batch (8 batches -> 8 NeuronCores). The
per-batch camera-response curve interp is evaluated exactly as a max-basis
ladder:

    interp(x, E, c) = C0 + sum_k g_k * max(x, E_k)

where g_k are the slope jumps of the piecewise-linear CRF at the breakpoints
E_k (g_k telescopes the segment slopes, 0 outside [E_0, E_255], so the sum
has no residual linear term) and C0 = c_0 - sum_k g_k E_k. Each term is one
DVE tensor_scalar pass (op0=max with the shared literal E_k, op1=mult with a
per-core [P,1] scalar read from a per-batch weight tile - the weight carries
the sign, so the same SPMD instruction stream serves all batches), plus one
tensor_tensor add into the accumulator.

The final clip(acc + C0, 0, 1) is two more tensor_scalar passes.

This walrus build has several codegen gaps worked around below:
 - the EventSemaphore butterfly barrier at TileContext tail doesn't compile
   -> replaced with plain per-engine DRAINs;
 - any instruction with >=2 sem waits fails setupSyncWait -> extra waits are
   split onto same-engine TensorCopy carriers; DMAs are kept to a single wait
   by full-tile DVE "touch" copies before each slot reuse;
 - static DMAs are pinned to the SP queue.
"""
import hashlib
import numpy as np

B, C, H, W = 8, 3, 1080, 1920
K = 256
NPIX = C * H * W            # 6,220,800 per batch
P = 128
F = NPIX // P               # 48,600 per partition
NPH = 4                     # mega-phases
CH = F // NPH               # 12,150 per phase

_cache = {}


def _patch_toolchain():
    import concourse.bass_utils as bu
    from concourse.tile import TileContext

    def patched_dab(self, tick_clock, wait_clock):
        for eng in self.nc.engines.values():
            eng.drain()
        popped = self.nc._tile_sem_poison_stack.pop()
        assert popped is self._sem_poison
    TileContext._drain_and_barrier = patched_dab

    if not getattr(bu.run_command, "_dma_flag_patched", False):
        orig = bu.run_command

        def patched(argv, **kw):
            argv = ["--assign-static-dmas-to-sp=true"
                    if a == "--assign-static-dmas-to-sp=false" else a for a in argv]
            return orig(argv, **kw)

        patched._dma_flag_patched = True
        bu.run_command = patched


def _fix_multiwait(nc):
    import concourse.mybir as mybir
    scr = nc.alloc_sbuf_tensor("multiwait_scr", [128, 1], mybir.dt.float32)
    cnt = [0]
    for fn in nc.m.functions:
        for blk in fn.blocks:
            out = []
            for inst in blk.instructions:
                si = inst.sync_info
                waits = list(si.on_wait) if (si and si.on_wait) else []
                if len(waits) > 1:
                    if inst.opcode in ("DMACopy", "DMA"):
                        eng_waits = [w for w in waits if not w.ant_name.startswith("DMAHW")]
                        si.on_wait = eng_waits[-1:] if eng_waits else waits[-1:]
                    else:
                        for w in waits[:-1]:
                            cnt[0] += 1
                            eng = nc.engines[inst.engine]
                            carrier = mybir.InstTensorCopy(
                                name=f"mwfix-{cnt[0]}",
                                ins=[eng.lower_ap(scr.ap())],
                                outs=[eng.lower_ap(scr.ap())],
                            )
                            carrier.engine = inst.engine
                            carrier.sync_info = mybir.SyncInfo(on_wait=[w], on_update=[])
                            out.append(carrier)
                            nc.register_instruction(carrier, overwrite=True)
                        si.on_wait = waits[-1:]
                out.append(inst)
            blk.instructions[:] = out


def _build(E32, nonce):
    """Build + jit the SPMD kernel. E32: the shared fp32 breakpoints (baked
    as instruction literals). Returns (fn, meta) where fn takes concatenated
    (x, gtab, nonce, zero-out) global arrays."""
    import jax
    import jax.numpy as jnp  # noqa: F401
    from jax.sharding import Mesh, PartitionSpec
    from jax.experimental.shard_map import shard_map
    import concourse.bass as bass
    import concourse.mybir as mybir
    from concourse.tile import TileContext
    from concourse.bass2jax import _bass_exec_p, install_neuronx_cc_hook, partition_id_tensor

    _patch_toolchain()

    nc = bass.Bass("TRN2", target_bir_lowering=False, debug=False)
    nc.declare_dram_parameter("cache_nonce", [1, 1 + nonce], mybir.dt.float32, isOutput=False)
    x = nc.declare_dram_parameter("x", [P, F], mybir.dt.float32, isOutput=False)
    gt = nc.declare_dram_parameter("gtab", [P, K + 1], mybir.dt.float32, isOutput=False)
    y = nc.declare_dram_parameter("y", [P, F], mybir.dt.float32, isOutput=True)

    Emax = mybir.AluOpType.max
    Emin = mybir.AluOpType.min
    Emul = mybir.AluOpType.mult
    Eadd = mybir.AluOpType.add

    with TileContext(nc) as tc:
        with tc.tile_pool(name="sbuf", bufs=1) as pool:
            gtt = pool.tile([P, K + 1], mybir.dt.float32, tag="gt", name="gtt")
            xt = pool.tile([P, CH], mybir.dt.float32, tag="xt", name="xt")
            acc = pool.tile([P, CH], mybir.dt.float32, tag="acc", name="acc")
            tmp0 = pool.tile([P, CH], mybir.dt.float32, tag="t0", name="tmp0")
            tmp1 = pool.tile([P, CH], mybir.dt.float32, tag="t1", name="tmp1")
            tmps = [tmp0, tmp1]
            nc.sync.dma_start(out=gtt[:], in_=gt[:])
            for p in range(NPH):
                sl = slice(p * CH, (p + 1) * CH)
                if p > 0:
                    # full-tile DVE touches: make the DVE the last accessor of
                    # both slots so the phase DMAs need only one sem wait.
                    nc.vector.tensor_copy(out=xt[:], in_=xt[:])
                    nc.vector.tensor_copy(out=acc[:], in_=acc[:])
                nc.sync.dma_start(out=xt[:], in_=x[:, sl])
                # k = 0 writes acc directly
                nc.vector.tensor_scalar(out=acc[:], in0=xt[:],
                                        scalar1=float(E32[0]), scalar2=gtt[:, 0:1],
                                        op0=Emax, op1=Emul)
                for k in range(1, K):
                    t = tmps[k % 2]
                    nc.vector.tensor_scalar(out=t[:], in0=xt[:],
                                            scalar1=float(E32[k]), scalar2=gtt[:, k:k + 1],
                                            op0=Emax, op1=Emul)
                    nc.vector.tensor_tensor(acc[:], acc[:], t[:], Eadd)
                # out = clip(acc + C0, 0, 1)
                nc.vector.tensor_scalar(out=acc[:], in0=acc[:],
                                        scalar1=gtt[:, K:K + 1], scalar2=0.0,
                                        op0=Eadd, op1=Emax)
                nc.vector.tensor_scalar(out=acc[:], in0=acc[:],
                                        scalar1=1.0, scalar2=None, op0=Emin)
                nc.sync.dma_start(out=y[:, sl], in_=acc[:])
    _fix_multiwait(nc)

    install_neuronx_cc_hook()
    partition_name = nc.partition_id_tensor.name if nc.partition_id_tensor else None
    in_names, out_names, out_avals = [], [], []
    for alloc in nc.m.functions[0].allocations:
        if not isinstance(alloc, mybir.MemoryLocationSet):
            continue
        name = alloc.memorylocations[0].name
        if alloc.kind == "ExternalInput":
            if name != partition_name:
                in_names.append(name)
        elif alloc.kind == "ExternalOutput":
            out_names.append(name)
            out_avals.append(jax.core.ShapedArray(tuple(alloc.tensor_shape),
                                                  mybir.dt.np(alloc.dtype)))
    all_in_names = list(in_names) + list(out_names)
    if partition_name is not None:
        all_in_names.append(partition_name)

    def _body(*args):
        operands = list(args)
        if partition_name is not None:
            operands.append(partition_id_tensor())
        return tuple(_bass_exec_p.bind(
            *operands, out_avals=tuple(out_avals), in_names=tuple(all_in_names),
            out_names=tuple(out_names), lowering_input_output_aliases=(),
            sim_require_finite=True, sim_require_nnan=True, nc=nc))

    devices = jax.devices()[:B]
    mesh = Mesh(np.asarray(devices), ("core",))
    n_ops = len(in_names) + len(out_names)
    fn = jax.jit(shard_map(_body, mesh=mesh,
                           in_specs=(PartitionSpec("core"),) * n_ops,
                           out_specs=(PartitionSpec("core"),) * len(out_names),
                           check_rep=False),
                 keep_unused=True)
    return fn, in_names, out_names


def _tables(E, f0, Hb, w):
    """Per-batch max-basis weights, computed in float64 on the host from the
    tiny K-sized inputs. Returns gtab [B, P, K+1] fp32."""
    E64 = E.astype(np.float64)
    gtab = np.zeros((B, P, K + 1), np.float32)
    for b in range(B):
        c = (f0.astype(np.float64) + Hb.astype(np.float64) @ w[b].astype(np.float64))
        slopes = np.diff(c) / np.diff(E64)
        g = np.diff(np.concatenate([[0.0], slopes, [0.0]]))
        C0 = c[0] - np.sum(g * E64)
        row = np.concatenate([g, [C0]]).astype(np.float32)
        gtab[b, :, :] = row[None, :]
    return gtab


def kernel(hdr_image, weights_w, E_samples, f0_mean, H_basis):
    import jax
    hdr_image = np.asarray(hdr_image, dtype=np.float32)
    weights_w = np.asarray(weights_w, dtype=np.float32)
    E_samples = np.asarray(E_samples, dtype=np.float32)
    f0_mean = np.asarray(f0_mean, dtype=np.float32)
    H_basis = np.asarray(H_basis, dtype=np.float32)

    key = hashlib.sha256(E_samples.tobytes()).hexdigest()
    nonce = (int(key[:8], 16) % 911) + 1
    if key not in _cache:
        _cache[key] = _build(E_samples, nonce)
    fn, in_names, out_names = _cache[key]

    gtab = _tables(E_samples, f0_mean, H_basis, weights_w)
    xs = hdr_image.reshape(B, P, F)

    per_core_inputs = {
        "x": xs,                                              # [B, P, F]
        "gtab": gtab,                                         # [B, P, K+1]
        "cache_nonce": np.zeros((B, 1, 1 + nonce), np.float32),
    }
    concat_in = [per_core_inputs[name].reshape(B * per_core_inputs[name].shape[1],
                                               *per_core_inputs[name].shape[2:])
                 for name in in_names]
    concat_zero = [np.zeros((B * P, F), np.float32)]

    out = fn(*concat_in, *concat_zero)
    res = np.asarray(out[0]).reshape(B, P, F)
    return res.reshape(B, C, H, W).astype(np.float32)


if __name__ == "__main__":
    rng = np.random.default_rng(0)
    demo = {
        "hdr_image": rng.random((B, C, H, W), np.float32),
        "weights_w": (rng.standard_normal((B, 25)) * 0.1).astype(np.float32),
        "E_samples": np.sort(rng.random(K).astype(np.float32)),
        "f0_mean": np.linspace(0, 1, K, dtype=np.float32),
        "H_basis": (rng.standard_normal((K, 25)) * 0.05).astype(np.float32),
    }
    out = kernel(**demo)
    print("kernel output", out.shape, out.dtype, out.min(), out.max())
